# revision 4
# baseline (speedup 1.0000x reference)
# Trainium2 Bass kernel for nn_CustomImageCosineSimLoss (N=4096, D=512, 8 cores).
#
# Sharding: image/text rows data-parallel across 8 cores (512 local rows each);
# full text replicated. Host does O(N*D) prep (fp8 casts, normalization,
# one-hots, group sums); all O(N^2) pair work runs on device.
#
# Math per core (L=512 local rows i, all N=4096 cols j, G=64 groups):
#   sweep1: sim = t8_loc^T @ t8        (fp8 DoubleRow matmuls -> PSUM)
#           plain PSUM->bf16 drain (Scalar); row-min (DVE) -> mn_i
#           mx_i = ||t_i||^2 (STT); s_i = (mx-mn+eps)/8
#   simmn = sim - mn_i (bias-only pass, split Scalar/Pool)
#   sweep2: t1 = 8*ihat^T @ that - BIG*onehot   (8*cos + aligned mask, PSUM)
#           fused STT from PSUM: Mxe_i = sum_j max(s_i*t1, simmn)  (DVE)
#   identity: relu(s*t1 + mn - sim) = max(s*t1, sim-mn) - (sim-mn), so
#     sum_j 8*relu(cos-w)*s = Mxe_i - S_i + 4096*mn_i,  S_i = sum_j sim_ij
#     = t_i . T8 (T8 = sum_j f8(t_j), host). Aligned pairs: -BIG*s forces
#     max = simmn, cancelling exactly against -S+4096mn.
#   aligned part: G1 - sum_aligned cos; cos sum = Frobenius(U8, Vg)/8 with
#     U8 = oh_loc @ ihat8 (4 device matmuls), Vg = group sums of that (host).
import numpy as np
import ml_dtypes

import concourse.mybir as mybir
import concourse.tile as tile
from concourse import bacc
from concourse.bass import ts

BF16 = mybir.dt.bfloat16
F32 = mybir.dt.float32
F8 = mybir.dt.float8e4
AF = mybir.ActivationFunctionType
OP = mybir.AluOpType
PM = mybir.MatmulPerfMode
nbf = ml_dtypes.bfloat16

N, D, G, NCORES = 4096, 512, 64, 8
L = N // NCORES            # 512 local rows per core
KT = D // 128              # 4 contraction chunks of 128
IT = L // 128              # 4 local i-tiles
JT = N // 512              # 8 j-tiles
BIG = 240.0                # exact in fp8-e4m3; dominates 8*cos
EPS_W = 1e-6

_CACHE = {}


def _build_program():
    nc = bacc.Bacc("TRN2", target_bir_lowering=False, debug=False,
                   enable_asserts=True, num_devices=NCORES)

    d_txt8_T = nc.dram_tensor("txt8_T", [D, N], F8, kind="ExternalInput").ap()
    d_txt8_T_loc = nc.dram_tensor("txt8_T_loc", [D, L], F8, kind="ExternalInput").ap()
    d_that8_T = nc.dram_tensor("that8_T", [D, N], F8, kind="ExternalInput").ap()
    d_ihat8_T = nc.dram_tensor("ihat8_T", [D, L], F8, kind="ExternalInput").ap()
    d_ihat8_rows = nc.dram_tensor("ihat8_rows", [L, D], F8, kind="ExternalInput").ap()
    d_txtloc = nc.dram_tensor("txtloc_rows", [L, D], BF16, kind="ExternalInput").ap()
    d_T8 = nc.dram_tensor("T8_bcast", [128, D], F32, kind="ExternalInput").ap()
    d_oh_iT = nc.dram_tensor("oh_iT", [L, G], F8, kind="ExternalInput").ap()
    d_mask_lhsT = nc.dram_tensor("mask_lhsT", [256, L], F8, kind="ExternalInput").ap()
    d_mask_rhsT = nc.dram_tensor("mask_rhsT", [256, N], F8, kind="ExternalInput").ap()
    d_Vg = nc.dram_tensor("Vg", [G, D], BF16, kind="ExternalInput").ap()
    d_partials = nc.dram_tensor("partials", [128, 24], F32, kind="ExternalOutput").ap()

    with tile.TileContext(nc) as tc:
        with (
            tc.tile_pool(name="persist", bufs=1) as pp,
            tc.tile_pool(name="sims", bufs=IT) as psim,
            tc.tile_pool(name="simmns", bufs=2) as psmn,
            tc.tile_pool(name="small", bufs=1) as psm,
            tc.tile_pool(name="stats", bufs=2) as pst,
            tc.tile_pool(name="psum", bufs=2, space="PSUM") as pps,
        ):
            # ---------------- loads (16-way split on the big sweeps) --------
            txt8_T = pp.tile([128, KT * N], F8)
            txt8_T_v = txt8_T[:].rearrange("p (c j) -> p c j", c=KT)
            d_txt8_T_v = d_txt8_T.rearrange("(c p) j -> p c j", p=128)
            for jt in range(JT):
                for ph in range(2):
                    sp = slice(64 * ph, 64 * ph + 64)
                    nc.sync.dma_start(txt8_T_v[sp, :, ts(jt, 512)],
                                      d_txt8_T_v[sp, :, ts(jt, 512)])

            txt8_T_loc = pp.tile([128, KT * L], F8)
            nc.sync.dma_start(txt8_T_loc[:].rearrange("p (c i) -> p c i", c=KT),
                              d_txt8_T_loc.rearrange("(c p) i -> p c i", p=128))
            txt8_T_loc_v = txt8_T_loc[:].rearrange("p (c i) -> p c i", c=KT)

            txtloc_rows = pp.tile([128, IT * D], BF16)
            nc.sync.dma_start(txtloc_rows[:].rearrange("p (t d) -> p t d", t=IT),
                              d_txtloc.rearrange("(t p) d -> p t d", p=128))
            txtloc_v = txtloc_rows[:].rearrange("p (t d) -> p t d", t=IT)

            T8_sb = pp.tile([128, D], F32)
            nc.sync.dma_start(T8_sb[:], d_T8)

            that8_T = pp.tile([128, KT * N], F8)
            that8_T_v = that8_T[:].rearrange("p (c j) -> p c j", c=KT)
            d_that8_T_v = d_that8_T.rearrange("(c p) j -> p c j", p=128)
            for jt in range(JT):
                for ph in range(2):
                    sp = slice(64 * ph, 64 * ph + 64)
                    nc.sync.dma_start(that8_T_v[sp, :, ts(jt, 512)],
                                      d_that8_T_v[sp, :, ts(jt, 512)])

            ihat8_T = pp.tile([128, KT * L], F8)
            nc.sync.dma_start(ihat8_T[:].rearrange("p (c i) -> p c i", c=KT),
                              d_ihat8_T.rearrange("(c p) i -> p c i", p=128))
            ihat8_T_v = ihat8_T[:].rearrange("p (c i) -> p c i", c=KT)

            ihat8_rows = pp.tile([128, IT * D], F8)
            nc.sync.dma_start(ihat8_rows[:].rearrange("p (t d) -> p t d", t=IT),
                              d_ihat8_rows.rearrange("(t p) d -> p t d", p=128))
            ihat8_rows_v = ihat8_rows[:].rearrange("p (t d) -> p t d", t=IT)

            oh_iT = pp.tile([128, IT * G], F8)
            nc.sync.dma_start(oh_iT[:].rearrange("p (t g) -> p t g", t=IT),
                              d_oh_iT.rearrange("(t p) g -> p t g", p=128))
            oh_iT_v = oh_iT[:].rearrange("p (t g) -> p t g", t=IT)

            mask_lhsT = pp.tile([128, 2 * L], F8)
            nc.sync.dma_start(mask_lhsT[:].rearrange("p (b i) -> p b i", b=2),
                              d_mask_lhsT.rearrange("(b p) i -> p b i", p=128))
            mask_lhsT_v = mask_lhsT[:].rearrange("p (b i) -> p b i", b=2)
            mask_rhsT = pp.tile([128, 2 * N], F8)
            nc.sync.dma_start(mask_rhsT[:].rearrange("p (b j) -> p b j", b=2),
                              d_mask_rhsT.rearrange("(b p) j -> p b j", p=128))
            mask_rhsT_v = mask_rhsT[:].rearrange("p (b j) -> p b j", b=2)

            Vg_sb = pp.tile([G, D], BF16)
            nc.sync.dma_start(Vg_sb[:], d_Vg)

            partials = pp.tile([128, 24], F32)
            nc.gpsimd.memset(partials[:], 0.0)

            # ---------- early row stats (overlap with DMA) ----------
            # mx_i = ||t_i||^2 ; S_i = t_i . T8 = sum_j sim_ij
            mx = psm.tile([128, IT], F32)
            for t in range(IT):
                junk = pst.tile([128, D], BF16, tag="junk")
                nc.vector.scalar_tensor_tensor(
                    out=junk[:], in0=txtloc_v[:, t, :], scalar=1.0,
                    in1=txtloc_v[:, t, :], op0=OP.mult, op1=OP.mult,
                    accum_out=mx[:, t:t + 1])
            for t in range(IT):
                junk = pst.tile([128, D], BF16, tag="junk")
                nc.vector.scalar_tensor_tensor(
                    out=junk[:], in0=txtloc_v[:, t, :], scalar=1.0,
                    in1=T8_sb[:], op0=OP.mult, op1=OP.mult,
                    accum_out=partials[:, 8 + t:9 + t])

            # ---------------- sweep 1: sim, plain drains, row-min -----------
            sim_panels = []
            mins32 = psm.tile([128, IT * 2], F32)
            mn32 = psm.tile([128, IT], F32)
            negmn = psm.tile([128, IT], F32)
            s32 = psm.tile([128, IT], F32)
            tmp32 = psm.tile([128, IT], F32)
            for it in range(IT):
                sim_sb = psim.tile([128, N], BF16, tag="sim")
                for h in range(2):
                    ps = pps.tile([128, 2048], F32, tag="mm")
                    for jq in range(4):
                        jt = 4 * h + jq
                        for cp in range(KT // 2):
                            nc.tensor.matmul(
                                ps[:, ts(jq, 512)],
                                txt8_T_loc_v[:, 2 * cp:2 * cp + 2, ts(it, 128)],
                                txt8_T_v[:, 2 * cp:2 * cp + 2, ts(jt, 512)],
                                start=(cp == 0), stop=(cp == KT // 2 - 1),
                                perf_mode=PM.DoubleRow)
                    nc.scalar.activation(out=sim_sb[:, ts(h, 2048)], in_=ps[:],
                                         func=AF.Identity)
                    nc.vector.tensor_reduce(
                        out=mins32[:, it * 2 + h:it * 2 + h + 1],
                        in_=sim_sb[:, ts(h, 2048)],
                        axis=mybir.AxisListType.X, op=OP.min)
                nc.vector.tensor_reduce(
                    out=mn32[:, it:it + 1], in_=mins32[:, it * 2:it * 2 + 2],
                    axis=mybir.AxisListType.X, op=OP.min)
                nc.vector.tensor_scalar_mul(out=negmn[:, it:it + 1],
                                            in0=mn32[:, it:it + 1], scalar1=-1.0)
                nc.vector.tensor_tensor(out=tmp32[:, it:it + 1],
                                        in0=mx[:, it:it + 1],
                                        in1=mn32[:, it:it + 1], op=OP.subtract)
                nc.vector.tensor_scalar(out=s32[:, it:it + 1],
                                        in0=tmp32[:, it:it + 1],
                                        scalar1=EPS_W, scalar2=0.125,
                                        op0=OP.add, op1=OP.mult)
                sim_panels.append(sim_sb)

            # ---------------- U8 = oh_loc @ ihat8 ; Frobenius with Vg -------
            u8ps = pps.tile([128, 2048], F32, tag="mm")
            for t in range(IT):
                nc.tensor.matmul(u8ps[0:G, 0:D], oh_iT_v[:, t, :],
                                 ihat8_rows_v[:, t, :],
                                 start=(t == 0), stop=(t == IT - 1))
            junk64 = pst.tile([G, D], BF16, tag="junk64")
            nc.vector.scalar_tensor_tensor(
                out=junk64[:], in0=u8ps[0:G, 0:D], scalar=1.0,
                in1=Vg_sb[:], op0=OP.mult, op1=OP.mult,
                accum_out=partials[0:G, 20:21])

            # ---------------- sweep 2: masked 8cos, fused max-from-PSUM -----
            junkpan = pp.tile([128, N], BF16)
            for it in range(IT):
                # simmn = sim - mn_i: one half on Scalar, one half on Pool
                simmn = psmn.tile([128, N], BF16, tag="simmn")
                nc.scalar.activation(out=simmn[:, 0:2048],
                                     in_=sim_panels[it][:, 0:2048],
                                     func=AF.Identity, bias=negmn[:, it:it + 1])
                nc.gpsimd.tensor_scalar(out=simmn[:, 2048:4096],
                                        in0=sim_panels[it][:, 2048:4096],
                                        scalar1=negmn[:, it:it + 1], scalar2=None,
                                        op0=OP.add)
                for h in range(2):
                    ps = pps.tile([128, 2048], F32, tag="mm")
                    for jq in range(4):
                        jt = 4 * h + jq
                        nc.tensor.matmul(ps[:, ts(jq, 512)],
                                         mask_lhsT_v[:, :, ts(it, 128)],
                                         mask_rhsT_v[:, :, ts(jt, 512)],
                                         start=True, stop=False,
                                         perf_mode=PM.DoubleRow)
                    for jq in range(4):
                        jt = 4 * h + jq
                        for cp in range(KT // 2):
                            nc.tensor.matmul(
                                ps[:, ts(jq, 512)],
                                ihat8_T_v[:, 2 * cp:2 * cp + 2, ts(it, 128)],
                                that8_T_v[:, 2 * cp:2 * cp + 2, ts(jt, 512)],
                                start=False, stop=(cp == KT // 2 - 1),
                                perf_mode=PM.DoubleRow)
                    # Mxe += sum_j max(s_i * t1, simmn), straight from PSUM
                    nc.vector.scalar_tensor_tensor(
                        out=junkpan[:, ts(h, 2048)], in0=ps[:],
                        scalar=s32[:, it:it + 1], in1=simmn[:, ts(h, 2048)],
                        op0=OP.mult, op1=OP.max,
                        accum_out=partials[:, 2 * it + h:2 * it + h + 1])

            # ship s and mn for the host-side reduction
            nc.vector.tensor_scalar_mul(out=partials[:, 12:16], in0=s32[:],
                                        scalar1=1.0)
            nc.vector.tensor_scalar_mul(out=partials[:, 16:20], in0=mn32[:],
                                        scalar1=1.0)

            nc.sync.dma_start(d_partials, partials[:])

    nc.compile()
    return nc


def _host_in_maps(image_features, text_features, instr_d):
    nf8 = mybir.dt.np(F8)
    img = np.asarray(image_features, np.float32)
    txt = np.asarray(text_features, np.float32)
    ins = np.asarray(instr_d)
    oh = (ins[None, :] == np.arange(G, dtype=ins.dtype)[:, None]).astype(np.float32)

    tn = np.sqrt((txt * txt).sum(1))
    inorm = np.sqrt((img * img).sum(1))
    that = txt / tn[:, None]
    ihat8 = (8.0 / inorm[:, None]) * img

    txt8 = txt.astype(nf8)
    txt8_T = np.ascontiguousarray(txt8.T)
    that8_T = np.ascontiguousarray(that.astype(nf8).T)
    ihat8_f8 = ihat8.astype(nf8)
    ihat8_T = np.ascontiguousarray(ihat8_f8.T)
    T8 = txt8.astype(np.float32).sum(0)                      # [D]
    T8_bcast = np.ascontiguousarray(
        np.broadcast_to(T8[None, :], (128, D)), dtype=np.float32)
    Vg = (oh @ that).astype(nbf)                             # [G, D]
    oh_iT_full = np.ascontiguousarray(oh.T).astype(nf8)

    # DoubleRow-padded mask operands: block 0 rows 0..63 hold the one-hots,
    # everything else zero. lhs carries the -BIG scale.
    mask_rhsT = np.zeros((256, N), nf8)
    mask_rhsT[0:G] = oh.astype(nf8)

    in_maps = []
    for c in range(NCORES):
        sl = slice(c * L, (c + 1) * L)
        mask_lhsT = np.zeros((256, L), nf8)
        mask_lhsT[0:G] = (-BIG * oh[:, sl]).astype(nf8)
        in_maps.append({
            "txt8_T": txt8_T,
            "txt8_T_loc": np.ascontiguousarray(txt8_T[:, sl]),
            "that8_T": that8_T,
            "ihat8_T": np.ascontiguousarray(ihat8_T[:, sl]),
            "ihat8_rows": np.ascontiguousarray(ihat8_f8[sl]),
            "txtloc_rows": np.ascontiguousarray(txt[sl].astype(nbf)),
            "T8_bcast": T8_bcast,
            "oh_iT": np.ascontiguousarray(oh_iT_full[sl]),
            "mask_lhsT": mask_lhsT,
            "mask_rhsT": mask_rhsT,
            "Vg": Vg,
        })
    return in_maps


def _reduce(partials_per_core, ins):
    ngg = (ins[None, :] == np.arange(G, dtype=ins.dtype)[:, None]).sum(1)  # [G]
    total = np.float64(0.0)
    for c, p in enumerate(partials_per_core):
        p = np.asarray(p, np.float64)
        ngl = (ins[c * L:(c + 1) * L][None, :]
               == np.arange(G, dtype=ins.dtype)[:, None]).sum(1)  # [G]
        Mxe = p[:, 0:8].reshape(128, 4, 2).sum(2)                # [128, IT]
        S = p[:, 8:12]
        s = p[:, 12:16]
        mn = p[:, 16:20]
        F8sum = p[0:G, 20].sum()
        relu8 = ((Mxe - S + N * mn) / s).sum()
        g1 = np.float64((ngl * ngg).sum())
        total += relu8 / 8.0 + g1 - F8sum / 8.0
    return np.float32(total / (N * N))


def kernel(**inputs) -> np.ndarray:
    from concourse.bass_utils import run_bass_kernel_spmd

    if "nc" not in _CACHE:
        _CACHE["nc"] = _build_program()
    nc = _CACHE["nc"]
    in_maps = _host_in_maps(**inputs)
    res = run_bass_kernel_spmd(nc, in_maps, core_ids=list(range(NCORES)),
                               trace=False)
    _CACHE["last_res"] = res

    ins = np.asarray(inputs["instr_d"])
    return _reduce([r["partials"] for r in res.results], ins)


# revision 5
# speedup vs baseline: 2.2466x; 2.2466x over previous
# Trainium2 Bass kernel for nn_CustomImageCosineSimLoss (N=4096, D=512, 8 cores).
#
# Sharding: image/text rows data-parallel across 8 cores (512 local rows each);
# full text replicated. Host does O(N*D) prep (fp8 casts, normalization,
# one-hots, group sums); all O(N^2) pair work runs on device.
#
# Math per core (L=512 local rows i, all N=4096 cols j, G=64 groups):
#   sweep1: sim = t8_loc^T @ t8        (fp8 DoubleRow matmuls -> PSUM)
#           plain PSUM->bf16 drain (Scalar); row-min (DVE) -> mn_i
#           mx_i = ||t_i||^2 (STT); s_i = (mx-mn+eps)/8
#   simmn = sim - mn_i (bias-only pass, split Scalar/Pool)
#   sweep2: t1 = 8*ihat^T @ that - BIG*onehot   (8*cos + aligned mask, PSUM)
#           fused STT from PSUM: Mxe_i = sum_j max(s_i*t1, simmn)  (DVE)
#   identity: relu(s*t1 + mn - sim) = max(s*t1, sim-mn) - (sim-mn), so
#     sum_j 8*relu(cos-w)*s = Mxe_i - S_i + 4096*mn_i,  S_i = sum_j sim_ij
#     = t_i . T8 (T8 = sum_j f8(t_j), host). Aligned pairs: -BIG*s forces
#     max = simmn, cancelling exactly against -S+4096mn.
#   aligned part: G1 - sum_aligned cos; cos sum = Frobenius(U8, Vg)/8 with
#     U8 = oh_loc @ ihat8 (4 device matmuls), Vg = group sums of that (host).
import numpy as np
import ml_dtypes

import concourse.mybir as mybir
import concourse.tile as tile
from concourse import bacc
from concourse.bass import ts

BF16 = mybir.dt.bfloat16
F32 = mybir.dt.float32
F8 = mybir.dt.float8e4
AF = mybir.ActivationFunctionType
OP = mybir.AluOpType
PM = mybir.MatmulPerfMode
nbf = ml_dtypes.bfloat16

N, D, G, NCORES = 4096, 512, 64, 8
L = N // NCORES            # 512 local rows per core
KT = D // 128              # 4 contraction chunks of 128
IT = L // 128              # 4 local i-tiles
JT = N // 512              # 8 j-tiles
BIG = 240.0                # exact in fp8-e4m3; dominates 8*cos
EPS_W = 1e-6

_CACHE = {}


def _build_program():
    nc = bacc.Bacc("TRN2", target_bir_lowering=False, debug=False,
                   enable_asserts=True, num_devices=NCORES)

    d_txt8_T = nc.dram_tensor("txt8_T", [D, N], F8, kind="ExternalInput").ap()
    d_txt8_T_loc = nc.dram_tensor("txt8_T_loc", [D, L], F8, kind="ExternalInput").ap()
    d_that8_T = nc.dram_tensor("that8_T", [D, N], F8, kind="ExternalInput").ap()
    d_ihat8_T = nc.dram_tensor("ihat8_T", [D, L], F8, kind="ExternalInput").ap()
    d_ihat8_rows = nc.dram_tensor("ihat8_rows", [L, D], F8, kind="ExternalInput").ap()
    d_txtloc = nc.dram_tensor("txtloc_rows", [L, D], BF16, kind="ExternalInput").ap()
    d_T8 = nc.dram_tensor("T8_bcast", [128, D], F32, kind="ExternalInput").ap()
    d_oh_iT = nc.dram_tensor("oh_iT", [L, G], F8, kind="ExternalInput").ap()
    d_mask_lhsT = nc.dram_tensor("mask_lhsT", [256, L], F8, kind="ExternalInput").ap()
    d_mask_rhsT = nc.dram_tensor("mask_rhsT", [256, N], F8, kind="ExternalInput").ap()
    d_Vg = nc.dram_tensor("Vg", [G, D], BF16, kind="ExternalInput").ap()
    d_partials = nc.dram_tensor("partials", [128, 24], F32, kind="ExternalOutput").ap()

    with tile.TileContext(nc) as tc:
        with (
            tc.tile_pool(name="persist", bufs=1) as pp,
            tc.tile_pool(name="sims", bufs=IT) as psim,
            tc.tile_pool(name="simmns", bufs=2) as psmn,
            tc.tile_pool(name="small", bufs=1) as psm,
            tc.tile_pool(name="stats", bufs=2) as pst,
            tc.tile_pool(name="psum", bufs=2, space="PSUM") as pps,
        ):
            # ---------------- loads (16-way split on the big sweeps) --------
            txt8_T = pp.tile([128, KT * N], F8)
            txt8_T_v = txt8_T[:].rearrange("p (c j) -> p c j", c=KT)
            d_txt8_T_v = d_txt8_T.rearrange("(c p) j -> p c j", p=128)
            for jt in range(JT):
                for ph in range(2):
                    sp = slice(64 * ph, 64 * ph + 64)
                    nc.sync.dma_start(txt8_T_v[sp, :, ts(jt, 512)],
                                      d_txt8_T_v[sp, :, ts(jt, 512)])

            txt8_T_loc = pp.tile([128, KT * L], F8)
            nc.sync.dma_start(txt8_T_loc[:].rearrange("p (c i) -> p c i", c=KT),
                              d_txt8_T_loc.rearrange("(c p) i -> p c i", p=128))
            txt8_T_loc_v = txt8_T_loc[:].rearrange("p (c i) -> p c i", c=KT)

            txtloc_rows = pp.tile([128, IT * D], BF16)
            nc.sync.dma_start(txtloc_rows[:].rearrange("p (t d) -> p t d", t=IT),
                              d_txtloc.rearrange("(t p) d -> p t d", p=128))
            txtloc_v = txtloc_rows[:].rearrange("p (t d) -> p t d", t=IT)

            T8_sb = pp.tile([128, D], F32)
            nc.sync.dma_start(T8_sb[:], d_T8)

            that8_T = pp.tile([128, KT * N], F8)
            that8_T_v = that8_T[:].rearrange("p (c j) -> p c j", c=KT)
            d_that8_T_v = d_that8_T.rearrange("(c p) j -> p c j", p=128)
            for jt in range(JT):
                for ph in range(2):
                    sp = slice(64 * ph, 64 * ph + 64)
                    nc.sync.dma_start(that8_T_v[sp, :, ts(jt, 512)],
                                      d_that8_T_v[sp, :, ts(jt, 512)])

            ihat8_T = pp.tile([128, KT * L], F8)
            nc.sync.dma_start(ihat8_T[:].rearrange("p (c i) -> p c i", c=KT),
                              d_ihat8_T.rearrange("(c p) i -> p c i", p=128))
            ihat8_T_v = ihat8_T[:].rearrange("p (c i) -> p c i", c=KT)

            ihat8_rows = pp.tile([128, IT * D], F8)
            nc.sync.dma_start(ihat8_rows[:].rearrange("p (t d) -> p t d", t=IT),
                              d_ihat8_rows.rearrange("(t p) d -> p t d", p=128))
            ihat8_rows_v = ihat8_rows[:].rearrange("p (t d) -> p t d", t=IT)

            oh_iT = pp.tile([128, IT * G], F8)
            nc.sync.dma_start(oh_iT[:].rearrange("p (t g) -> p t g", t=IT),
                              d_oh_iT.rearrange("(t p) g -> p t g", p=128))
            oh_iT_v = oh_iT[:].rearrange("p (t g) -> p t g", t=IT)

            mask_lhsT = pp.tile([128, 2 * L], F8)
            nc.sync.dma_start(mask_lhsT[:].rearrange("p (b i) -> p b i", b=2),
                              d_mask_lhsT.rearrange("(b p) i -> p b i", p=128))
            mask_lhsT_v = mask_lhsT[:].rearrange("p (b i) -> p b i", b=2)
            mask_rhsT = pp.tile([128, 2 * N], F8)
            nc.sync.dma_start(mask_rhsT[:].rearrange("p (b j) -> p b j", b=2),
                              d_mask_rhsT.rearrange("(b p) j -> p b j", p=128))
            mask_rhsT_v = mask_rhsT[:].rearrange("p (b j) -> p b j", b=2)

            Vg_sb = pp.tile([G, D], BF16)
            nc.sync.dma_start(Vg_sb[:], d_Vg)

            partials = pp.tile([128, 24], F32)
            nc.gpsimd.memset(partials[:], 0.0)

            # ---------- early row stats (overlap with DMA) ----------
            # mx_i = ||t_i||^2 ; S_i = t_i . T8 = sum_j sim_ij
            mx = psm.tile([128, IT], F32)
            for t in range(IT):
                junk = pst.tile([128, D], BF16, tag="junk")
                nc.vector.scalar_tensor_tensor(
                    out=junk[:], in0=txtloc_v[:, t, :], scalar=1.0,
                    in1=txtloc_v[:, t, :], op0=OP.mult, op1=OP.mult,
                    accum_out=mx[:, t:t + 1])
            for t in range(IT):
                junk = pst.tile([128, D], BF16, tag="junk")
                nc.vector.scalar_tensor_tensor(
                    out=junk[:], in0=txtloc_v[:, t, :], scalar=1.0,
                    in1=T8_sb[:], op0=OP.mult, op1=OP.mult,
                    accum_out=partials[:, 8 + t:9 + t])

            # ---------------- sweep 1: sim, plain drains, row-min -----------
            sim_panels = []
            mins32 = psm.tile([128, IT * 2], F32)
            mn32 = psm.tile([128, IT], F32)
            negmn = psm.tile([128, IT], F32)
            s32 = psm.tile([128, IT], F32)
            tmp32 = psm.tile([128, IT], F32)
            for it in range(IT):
                sim_sb = psim.tile([128, N], BF16, tag="sim")
                for h in range(2):
                    ps = pps.tile([128, 2048], F32, tag="mm")
                    for jq in range(4):
                        jt = 4 * h + jq
                        for cp in range(KT // 2):
                            nc.tensor.matmul(
                                ps[:, ts(jq, 512)],
                                txt8_T_loc_v[:, 2 * cp:2 * cp + 2, ts(it, 128)],
                                txt8_T_v[:, 2 * cp:2 * cp + 2, ts(jt, 512)],
                                start=(cp == 0), stop=(cp == KT // 2 - 1),
                                perf_mode=PM.DoubleRow)
                    nc.scalar.activation(out=sim_sb[:, ts(h, 2048)], in_=ps[:],
                                         func=AF.Identity)
                    nc.vector.tensor_reduce(
                        out=mins32[:, it * 2 + h:it * 2 + h + 1],
                        in_=sim_sb[:, ts(h, 2048)],
                        axis=mybir.AxisListType.X, op=OP.min)
                nc.vector.tensor_reduce(
                    out=mn32[:, it:it + 1], in_=mins32[:, it * 2:it * 2 + 2],
                    axis=mybir.AxisListType.X, op=OP.min)
                nc.vector.tensor_scalar_mul(out=negmn[:, it:it + 1],
                                            in0=mn32[:, it:it + 1], scalar1=-1.0)
                nc.vector.tensor_tensor(out=tmp32[:, it:it + 1],
                                        in0=mx[:, it:it + 1],
                                        in1=mn32[:, it:it + 1], op=OP.subtract)
                nc.vector.tensor_scalar(out=s32[:, it:it + 1],
                                        in0=tmp32[:, it:it + 1],
                                        scalar1=EPS_W, scalar2=0.125,
                                        op0=OP.add, op1=OP.mult)
                sim_panels.append(sim_sb)

            # ---------------- U8 = oh_loc @ ihat8 ; Frobenius with Vg -------
            u8ps = pps.tile([128, 2048], F32, tag="mm")
            for t in range(IT):
                nc.tensor.matmul(u8ps[0:G, 0:D], oh_iT_v[:, t, :],
                                 ihat8_rows_v[:, t, :],
                                 start=(t == 0), stop=(t == IT - 1))
            junk64 = pst.tile([G, D], BF16, tag="junk64")
            nc.vector.scalar_tensor_tensor(
                out=junk64[:], in0=u8ps[0:G, 0:D], scalar=1.0,
                in1=Vg_sb[:], op0=OP.mult, op1=OP.mult,
                accum_out=partials[0:G, 20:21])

            # ---------------- sweep 2: masked 8cos, fused max-from-PSUM -----
            junkpan = pp.tile([128, N], BF16)
            for it in range(IT):
                # simmn = sim - mn_i (bias-only Scalar passes)
                simmn = psmn.tile([128, N], BF16, tag="simmn")
                for h in range(2):
                    nc.scalar.activation(out=simmn[:, ts(h, 2048)],
                                         in_=sim_panels[it][:, ts(h, 2048)],
                                         func=AF.Identity,
                                         bias=negmn[:, it:it + 1])
                for h in range(2):
                    ps = pps.tile([128, 2048], F32, tag="mm")
                    for jq in range(4):
                        jt = 4 * h + jq
                        nc.tensor.matmul(ps[:, ts(jq, 512)],
                                         mask_lhsT_v[:, :, ts(it, 128)],
                                         mask_rhsT_v[:, :, ts(jt, 512)],
                                         start=True, stop=False,
                                         perf_mode=PM.DoubleRow)
                    for jq in range(4):
                        jt = 4 * h + jq
                        for cp in range(KT // 2):
                            nc.tensor.matmul(
                                ps[:, ts(jq, 512)],
                                ihat8_T_v[:, 2 * cp:2 * cp + 2, ts(it, 128)],
                                that8_T_v[:, 2 * cp:2 * cp + 2, ts(jt, 512)],
                                start=False, stop=(cp == KT // 2 - 1),
                                perf_mode=PM.DoubleRow)
                    # Mxe += sum_j max(s_i * t1, simmn), straight from PSUM
                    nc.vector.scalar_tensor_tensor(
                        out=junkpan[:, ts(h, 2048)], in0=ps[:],
                        scalar=s32[:, it:it + 1], in1=simmn[:, ts(h, 2048)],
                        op0=OP.mult, op1=OP.max,
                        accum_out=partials[:, 2 * it + h:2 * it + h + 1])

            # ship s and mn for the host-side reduction
            nc.vector.tensor_scalar_mul(out=partials[:, 12:16], in0=s32[:],
                                        scalar1=1.0)
            nc.vector.tensor_scalar_mul(out=partials[:, 16:20], in0=mn32[:],
                                        scalar1=1.0)

            nc.sync.dma_start(d_partials, partials[:])

    nc.compile()
    return nc


def _host_in_maps(image_features, text_features, instr_d):
    nf8 = mybir.dt.np(F8)
    img = np.asarray(image_features, np.float32)
    txt = np.asarray(text_features, np.float32)
    ins = np.asarray(instr_d)
    oh = (ins[None, :] == np.arange(G, dtype=ins.dtype)[:, None]).astype(np.float32)

    tn = np.sqrt((txt * txt).sum(1))
    inorm = np.sqrt((img * img).sum(1))
    that = txt / tn[:, None]
    ihat8 = (8.0 / inorm[:, None]) * img

    txt8 = txt.astype(nf8)
    txt8_T = np.ascontiguousarray(txt8.T)
    that8_T = np.ascontiguousarray(that.astype(nf8).T)
    ihat8_f8 = ihat8.astype(nf8)
    ihat8_T = np.ascontiguousarray(ihat8_f8.T)
    T8 = txt8.astype(np.float32).sum(0)                      # [D]
    T8_bcast = np.ascontiguousarray(
        np.broadcast_to(T8[None, :], (128, D)), dtype=np.float32)
    Vg = (oh @ that).astype(nbf)                             # [G, D]
    oh_iT_full = np.ascontiguousarray(oh.T).astype(nf8)

    # DoubleRow-padded mask operands: block 0 rows 0..63 hold the one-hots,
    # everything else zero. lhs carries the -BIG scale.
    mask_rhsT = np.zeros((256, N), nf8)
    mask_rhsT[0:G] = oh.astype(nf8)

    in_maps = []
    for c in range(NCORES):
        sl = slice(c * L, (c + 1) * L)
        mask_lhsT = np.zeros((256, L), nf8)
        mask_lhsT[0:G] = (-BIG * oh[:, sl]).astype(nf8)
        in_maps.append({
            "txt8_T": txt8_T,
            "txt8_T_loc": np.ascontiguousarray(txt8_T[:, sl]),
            "that8_T": that8_T,
            "ihat8_T": np.ascontiguousarray(ihat8_T[:, sl]),
            "ihat8_rows": np.ascontiguousarray(ihat8_f8[sl]),
            "txtloc_rows": np.ascontiguousarray(txt[sl].astype(nbf)),
            "T8_bcast": T8_bcast,
            "oh_iT": np.ascontiguousarray(oh_iT_full[sl]),
            "mask_lhsT": mask_lhsT,
            "mask_rhsT": mask_rhsT,
            "Vg": Vg,
        })
    return in_maps


def _reduce(partials_per_core, ins):
    ngg = (ins[None, :] == np.arange(G, dtype=ins.dtype)[:, None]).sum(1)  # [G]
    total = np.float64(0.0)
    for c, p in enumerate(partials_per_core):
        p = np.asarray(p, np.float64)
        ngl = (ins[c * L:(c + 1) * L][None, :]
               == np.arange(G, dtype=ins.dtype)[:, None]).sum(1)  # [G]
        Mxe = p[:, 0:8].reshape(128, 4, 2).sum(2)                # [128, IT]
        S = p[:, 8:12]
        s = p[:, 12:16]
        mn = p[:, 16:20]
        F8sum = p[0:G, 20].sum()
        relu8 = ((Mxe - S + N * mn) / s).sum()
        g1 = np.float64((ngl * ngg).sum())
        total += relu8 / 8.0 + g1 - F8sum / 8.0
    return np.float32(total / (N * N))


def kernel(**inputs) -> np.ndarray:
    from concourse.bass_utils import run_bass_kernel_spmd

    if "nc" not in _CACHE:
        _CACHE["nc"] = _build_program()
    nc = _CACHE["nc"]
    in_maps = _host_in_maps(**inputs)
    res = run_bass_kernel_spmd(nc, in_maps, core_ids=list(range(NCORES)),
                               trace=False)
    _CACHE["last_res"] = res

    ins = np.asarray(inputs["instr_d"])
    return _reduce([r["partials"] for r in res.results], ins)


# revision 6
# speedup vs baseline: 2.3316x; 1.0378x over previous
# Trainium2 Bass kernel for nn_CustomImageCosineSimLoss (N=4096, D=512, 8 cores).
#
# Sharding: image/text rows data-parallel across 8 cores (512 local rows each);
# full text replicated. Host does O(N*D) prep (fp8 casts, normalization,
# one-hots, group sums); all O(N^2) pair work runs on device.
#
# Math per core (L=512 local rows i, all N=4096 cols j, G=64 groups):
#   sweep1: sim = t8_loc^T @ t8        (fp8 DoubleRow matmuls -> PSUM)
#           row-min straight off PSUM (DVE) || plain PSUM->bf16 drain (Scalar)
#           mx_i = ||t_i||^2 (STT); s_i = (mx-mn+eps)/8
#   simmn = sim - mn_i (bias-only Scalar passes)
#   sweep2: t1 = 8*ihat^T @ that - BIG*onehot   (8*cos + aligned mask, PSUM)
#           fused STT from PSUM: Mxe_i = sum_j max(s_i*t1, simmn)  (DVE)
#   identity: relu(s*t1 + mn - sim) = max(s*t1, sim-mn) - (sim-mn), so
#     sum_j 8*relu(cos-w)*s = Mxe_i - S_i + 4096*mn_i,  S_i = sum_j sim_ij
#     = t_i . T8 (T8 = sum_j f8(t_j), host). Aligned pairs: -BIG*s forces
#     max = simmn, cancelling exactly against -S+4096mn.
#   aligned part: G1 - sum_aligned cos; cos sum = Frobenius(U8, Vg)/8 with
#     U8 = oh_loc @ ihat8 (4 device matmuls), Vg = group sums of that (host).
#
# Big rhs tensors use a (jt, c, j) blocked layout so input DMAs run with 2 KiB
# descriptors, split 16 ways across queues, issued from both SP and Act.
import numpy as np
import ml_dtypes

import concourse.mybir as mybir
import concourse.tile as tile
from concourse import bacc
from concourse.bass import ts

BF16 = mybir.dt.bfloat16
F32 = mybir.dt.float32
F8 = mybir.dt.float8e4
AF = mybir.ActivationFunctionType
OP = mybir.AluOpType
PM = mybir.MatmulPerfMode
nbf = ml_dtypes.bfloat16

N, D, G, NCORES = 4096, 512, 64, 8
L = N // NCORES            # 512 local rows per core
KT = D // 128              # 4 contraction chunks of 128
IT = L // 128              # 4 local i-tiles
JT = N // 512              # 8 j-tiles
BIG = 240.0                # exact in fp8-e4m3; dominates 8*cos
EPS_W = 1e-6

_CACHE = {}


def _build_program():
    nc = bacc.Bacc("TRN2", target_bir_lowering=False, debug=False,
                   enable_asserts=True, num_devices=NCORES)

    d_txt8 = nc.dram_tensor("txt8_blk", [128, JT * KT * 512], F8,
                            kind="ExternalInput").ap()
    d_that8 = nc.dram_tensor("that8_blk", [128, JT * KT * 512], F8,
                             kind="ExternalInput").ap()
    d_txt8_T_loc = nc.dram_tensor("txt8_T_loc", [D, L], F8, kind="ExternalInput").ap()
    d_ihat8_T = nc.dram_tensor("ihat8_T", [D, L], F8, kind="ExternalInput").ap()
    d_ihat8_rows = nc.dram_tensor("ihat8_rows", [L, D], F8, kind="ExternalInput").ap()
    d_txtloc = nc.dram_tensor("txtloc_rows", [L, D], BF16, kind="ExternalInput").ap()
    d_T8 = nc.dram_tensor("T8_bcast", [128, D], F32, kind="ExternalInput").ap()
    d_oh_iT = nc.dram_tensor("oh_iT", [L, G], F8, kind="ExternalInput").ap()
    d_mask_lhsT = nc.dram_tensor("mask_lhsT", [256, L], F8, kind="ExternalInput").ap()
    d_mask_rhsT = nc.dram_tensor("mask_rhsT", [128, 2 * N], F8,
                                 kind="ExternalInput").ap()
    d_Vg = nc.dram_tensor("Vg", [G, D], BF16, kind="ExternalInput").ap()
    d_partials = nc.dram_tensor("partials", [128, 24], F32, kind="ExternalOutput").ap()

    with tile.TileContext(nc) as tc:
        with (
            tc.tile_pool(name="persist", bufs=1) as pp,
            tc.tile_pool(name="sims", bufs=IT) as psim,
            tc.tile_pool(name="simmns", bufs=2) as psmn,
            tc.tile_pool(name="small", bufs=1) as psm,
            tc.tile_pool(name="stats", bufs=2) as pst,
            tc.tile_pool(name="psum", bufs=2, space="PSUM") as pps,
        ):
            # -------- loads: critical tensors first, split SP/Act issue ----
            txt8_T_loc = pp.tile([128, KT * L], F8)
            nc.sync.dma_start(txt8_T_loc[:].rearrange("p (c i) -> p c i", c=KT),
                              d_txt8_T_loc.rearrange("(c p) i -> p c i", p=128))
            txt8_T_loc_v = txt8_T_loc[:].rearrange("p (c i) -> p c i", c=KT)

            txtloc_rows = pp.tile([128, IT * D], BF16)
            nc.sync.dma_start(txtloc_rows[:].rearrange("p (t d) -> p t d", t=IT),
                              d_txtloc.rearrange("(t p) d -> p t d", p=128))
            txtloc_v = txtloc_rows[:].rearrange("p (t d) -> p t d", t=IT)

            T8_sb = pp.tile([128, D], F32)
            nc.sync.dma_start(T8_sb[:], d_T8)

            # txt8 in (jt, c, j) blocked layout: 16 chunks, 2KB descriptors
            txt8 = pp.tile([128, JT * KT * 512], F8)
            txt8_v = txt8[:].rearrange("p (jt c j) -> p jt c j", jt=JT, c=KT)
            d_txt8_v = d_txt8.rearrange("p (jt c j) -> p jt c j", jt=JT, c=KT)
            for jt in range(JT):
                for ph in range(2):
                    sp = slice(64 * ph, 64 * ph + 64)
                    eng = nc.sync if ph == 0 else nc.scalar
                    eng.dma_start(txt8_v[sp, jt], d_txt8_v[sp, jt])

            that8 = pp.tile([128, JT * KT * 512], F8)
            that8_v = that8[:].rearrange("p (jt c j) -> p jt c j", jt=JT, c=KT)
            d_that8_v = d_that8.rearrange("p (jt c j) -> p jt c j", jt=JT, c=KT)
            for jt in range(JT):
                for ph in range(2):
                    sp = slice(64 * ph, 64 * ph + 64)
                    eng = nc.sync if ph == 0 else nc.scalar
                    eng.dma_start(that8_v[sp, jt], d_that8_v[sp, jt])

            ihat8_T = pp.tile([128, KT * L], F8)
            nc.scalar.dma_start(ihat8_T[:].rearrange("p (c i) -> p c i", c=KT),
                                d_ihat8_T.rearrange("(c p) i -> p c i", p=128))
            ihat8_T_v = ihat8_T[:].rearrange("p (c i) -> p c i", c=KT)

            ihat8_rows = pp.tile([128, IT * D], F8)
            nc.scalar.dma_start(ihat8_rows[:].rearrange("p (t d) -> p t d", t=IT),
                                d_ihat8_rows.rearrange("(t p) d -> p t d", p=128))
            ihat8_rows_v = ihat8_rows[:].rearrange("p (t d) -> p t d", t=IT)

            oh_iT = pp.tile([128, IT * G], F8)
            nc.sync.dma_start(oh_iT[:].rearrange("p (t g) -> p t g", t=IT),
                              d_oh_iT.rearrange("(t p) g -> p t g", p=128))
            oh_iT_v = oh_iT[:].rearrange("p (t g) -> p t g", t=IT)

            mask_lhsT = pp.tile([128, 2 * L], F8)
            nc.sync.dma_start(mask_lhsT[:].rearrange("p (b i) -> p b i", b=2),
                              d_mask_lhsT.rearrange("(b p) i -> p b i", p=128))
            mask_lhsT_v = mask_lhsT[:].rearrange("p (b i) -> p b i", b=2)
            mask_rhsT = pp.tile([128, 2 * N], F8)
            nc.sync.dma_start(mask_rhsT[:], d_mask_rhsT)
            mask_rhsT_v = mask_rhsT[:].rearrange("p (b j) -> p b j", b=2)

            Vg_sb = pp.tile([G, D], BF16)
            nc.sync.dma_start(Vg_sb[:], d_Vg)

            partials = pp.tile([128, 24], F32)
            nc.gpsimd.memset(partials[:], 0.0)

            # ---------- early row stats (overlap with DMA) ----------
            mx = psm.tile([128, IT], F32)
            for t in range(IT):
                junk = pst.tile([128, D], BF16, tag="junk")
                nc.vector.scalar_tensor_tensor(
                    out=junk[:], in0=txtloc_v[:, t, :], scalar=1.0,
                    in1=txtloc_v[:, t, :], op0=OP.mult, op1=OP.mult,
                    accum_out=mx[:, t:t + 1])
            for t in range(IT):
                junk = pst.tile([128, D], BF16, tag="junk")
                nc.vector.scalar_tensor_tensor(
                    out=junk[:], in0=txtloc_v[:, t, :], scalar=1.0,
                    in1=T8_sb[:], op0=OP.mult, op1=OP.mult,
                    accum_out=partials[:, 8 + t:9 + t])

            # ---------------- sweep 1: sim, PSUM row-min || plain drain ----
            sim_panels = []
            mins32 = psm.tile([128, IT * 2], F32)
            mn32 = psm.tile([128, IT], F32)
            negmn = psm.tile([128, IT], F32)
            s32 = psm.tile([128, IT], F32)
            tmp32 = psm.tile([128, IT], F32)
            for it in range(IT):
                sim_sb = psim.tile([128, N], BF16, tag="sim")
                for h in range(2):
                    ps = pps.tile([128, 2048], F32, tag="mm")
                    for jq in range(4):
                        jt = 4 * h + jq
                        for cp in range(KT // 2):
                            nc.tensor.matmul(
                                ps[:, ts(jq, 512)],
                                txt8_T_loc_v[:, 2 * cp:2 * cp + 2, ts(it, 128)],
                                txt8_v[:, jt, 2 * cp:2 * cp + 2, :],
                                start=(cp == 0), stop=(cp == KT // 2 - 1),
                                perf_mode=PM.DoubleRow)
                    nc.vector.tensor_reduce(
                        out=mins32[:, it * 2 + h:it * 2 + h + 1], in_=ps[:],
                        axis=mybir.AxisListType.X, op=OP.min)
                    nc.scalar.activation(out=sim_sb[:, ts(h, 2048)], in_=ps[:],
                                         func=AF.Identity)
                nc.vector.tensor_reduce(
                    out=mn32[:, it:it + 1], in_=mins32[:, it * 2:it * 2 + 2],
                    axis=mybir.AxisListType.X, op=OP.min)
                nc.vector.tensor_scalar_mul(out=negmn[:, it:it + 1],
                                            in0=mn32[:, it:it + 1], scalar1=-1.0)
                nc.vector.tensor_tensor(out=tmp32[:, it:it + 1],
                                        in0=mx[:, it:it + 1],
                                        in1=mn32[:, it:it + 1], op=OP.subtract)
                nc.vector.tensor_scalar(out=s32[:, it:it + 1],
                                        in0=tmp32[:, it:it + 1],
                                        scalar1=EPS_W, scalar2=0.125,
                                        op0=OP.add, op1=OP.mult)
                sim_panels.append(sim_sb)

            # ---------------- U8 = oh_loc @ ihat8 ; Frobenius with Vg -------
            u8ps = pps.tile([128, 2048], F32, tag="mm")
            for t in range(IT):
                nc.tensor.matmul(u8ps[0:G, 0:D], oh_iT_v[:, t, :],
                                 ihat8_rows_v[:, t, :],
                                 start=(t == 0), stop=(t == IT - 1))
            junk64 = pst.tile([G, D], BF16, tag="junk64")
            nc.vector.scalar_tensor_tensor(
                out=junk64[:], in0=u8ps[0:G, 0:D], scalar=1.0,
                in1=Vg_sb[:], op0=OP.mult, op1=OP.mult,
                accum_out=partials[0:G, 20:21])

            # ---------------- sweep 2: masked 8cos, fused max-from-PSUM -----
            junkpan = pp.tile([128, N], BF16)
            for it in range(IT):
                simmn = psmn.tile([128, N], BF16, tag="simmn")
                for h in range(2):
                    nc.scalar.activation(out=simmn[:, ts(h, 2048)],
                                         in_=sim_panels[it][:, ts(h, 2048)],
                                         func=AF.Identity,
                                         bias=negmn[:, it:it + 1])
                for h in range(2):
                    ps = pps.tile([128, 2048], F32, tag="mm")
                    for jq in range(4):
                        jt = 4 * h + jq
                        nc.tensor.matmul(ps[:, ts(jq, 512)],
                                         mask_lhsT_v[:, :, ts(it, 128)],
                                         mask_rhsT_v[:, :, ts(jt, 512)],
                                         start=True, stop=False,
                                         perf_mode=PM.DoubleRow)
                    for jq in range(4):
                        jt = 4 * h + jq
                        for cp in range(KT // 2):
                            nc.tensor.matmul(
                                ps[:, ts(jq, 512)],
                                ihat8_T_v[:, 2 * cp:2 * cp + 2, ts(it, 128)],
                                that8_v[:, jt, 2 * cp:2 * cp + 2, :],
                                start=False, stop=(cp == KT // 2 - 1),
                                perf_mode=PM.DoubleRow)
                    # Mxe += sum_j max(s_i * t1, simmn), straight from PSUM
                    nc.vector.scalar_tensor_tensor(
                        out=junkpan[:, ts(h, 2048)], in0=ps[:],
                        scalar=s32[:, it:it + 1], in1=simmn[:, ts(h, 2048)],
                        op0=OP.mult, op1=OP.max,
                        accum_out=partials[:, 2 * it + h:2 * it + h + 1])

            # ship s and mn for the host-side reduction
            nc.vector.tensor_scalar_mul(out=partials[:, 12:16], in0=s32[:],
                                        scalar1=1.0)
            nc.vector.tensor_scalar_mul(out=partials[:, 16:20], in0=mn32[:],
                                        scalar1=1.0)

            nc.sync.dma_start(d_partials, partials[:])

    nc.compile()
    return nc


def _blocked(a_T):
    # [D, N] -> [128, (jt c j)] where element (p, jt, c, j) = a_T[c*128+p, jt*512+j]
    return np.ascontiguousarray(
        a_T.reshape(KT, 128, JT, 512).transpose(1, 2, 0, 3).reshape(128, -1))


def _host_in_maps(image_features, text_features, instr_d):
    nf8 = mybir.dt.np(F8)
    img = np.asarray(image_features, np.float32)
    txt = np.asarray(text_features, np.float32)
    ins = np.asarray(instr_d)
    oh = (ins[None, :] == np.arange(G, dtype=ins.dtype)[:, None]).astype(np.float32)

    tn = np.sqrt((txt * txt).sum(1))
    inorm = np.sqrt((img * img).sum(1))
    that = txt / tn[:, None]
    ihat8 = (8.0 / inorm[:, None]) * img

    txt8 = txt.astype(nf8)
    txt8_T = np.ascontiguousarray(txt8.T)
    txt8_blk = _blocked(txt8_T)
    that8_blk = _blocked(that.astype(nf8).T)
    ihat8_f8 = ihat8.astype(nf8)
    ihat8_T = np.ascontiguousarray(ihat8_f8.T)
    T8 = txt8.astype(np.float32).sum(0)                      # [D]
    T8_bcast = np.ascontiguousarray(
        np.broadcast_to(T8[None, :], (128, D)), dtype=np.float32)
    Vg = (oh @ that).astype(nbf)                             # [G, D]
    oh_iT_full = np.ascontiguousarray(oh.T).astype(nf8)

    # DoubleRow-padded mask operands: block 0 rows 0..63 hold the one-hots,
    # everything else zero. lhs carries the -BIG scale. rhs pre-blocked to
    # [128, (b, j)] so the DMA is a straight 2-row copy.
    mask_rhsT = np.zeros((2, 128, N), nf8)
    mask_rhsT[0, 0:G] = oh.astype(nf8)
    mask_rhsT_blk = np.ascontiguousarray(
        mask_rhsT.transpose(1, 0, 2).reshape(128, 2 * N))

    in_maps = []
    for c in range(NCORES):
        sl = slice(c * L, (c + 1) * L)
        mask_lhsT = np.zeros((256, L), nf8)
        mask_lhsT[0:G] = (-BIG * oh[:, sl]).astype(nf8)
        in_maps.append({
            "txt8_blk": txt8_blk,
            "txt8_T_loc": np.ascontiguousarray(txt8_T[:, sl]),
            "that8_blk": that8_blk,
            "ihat8_T": np.ascontiguousarray(ihat8_T[:, sl]),
            "ihat8_rows": np.ascontiguousarray(ihat8_f8[sl]),
            "txtloc_rows": np.ascontiguousarray(txt[sl].astype(nbf)),
            "T8_bcast": T8_bcast,
            "oh_iT": np.ascontiguousarray(oh_iT_full[sl]),
            "mask_lhsT": mask_lhsT,
            "mask_rhsT": mask_rhsT_blk,
            "Vg": Vg,
        })
    return in_maps


def _reduce(partials_per_core, ins):
    ngg = (ins[None, :] == np.arange(G, dtype=ins.dtype)[:, None]).sum(1)  # [G]
    total = np.float64(0.0)
    for c, p in enumerate(partials_per_core):
        p = np.asarray(p, np.float64)
        ngl = (ins[c * L:(c + 1) * L][None, :]
               == np.arange(G, dtype=ins.dtype)[:, None]).sum(1)  # [G]
        Mxe = p[:, 0:8].reshape(128, 4, 2).sum(2)                # [128, IT]
        S = p[:, 8:12]
        s = p[:, 12:16]
        mn = p[:, 16:20]
        F8sum = p[0:G, 20].sum()
        relu8 = ((Mxe - S + N * mn) / s).sum()
        g1 = np.float64((ngl * ngg).sum())
        total += relu8 / 8.0 + g1 - F8sum / 8.0
    return np.float32(total / (N * N))


def kernel(**inputs) -> np.ndarray:
    from concourse.bass_utils import run_bass_kernel_spmd

    if "nc" not in _CACHE:
        _CACHE["nc"] = _build_program()
    nc = _CACHE["nc"]
    in_maps = _host_in_maps(**inputs)
    res = run_bass_kernel_spmd(nc, in_maps, core_ids=list(range(NCORES)),
                               trace=False)
    _CACHE["last_res"] = res

    ins = np.asarray(inputs["instr_d"])
    return _reduce([r["partials"] for r in res.results], ins)


# revision 7
# speedup vs baseline: 2.5667x; 1.1008x over previous
# Trainium2 Bass kernel for nn_CustomImageCosineSimLoss (N=4096, D=512, 8 cores).
#
# Sharding: image/text rows data-parallel across 8 cores (512 local rows each);
# full text replicated. Host does O(N*D) prep (fp8 casts, normalization,
# one-hots, group sums); all O(N^2) pair work runs on device.
#
# Math per core (L=512 local rows i, all N=4096 cols j, G=64 groups):
#   sweep1: sim = t8_loc^T @ t8        (fp8 DoubleRow matmuls -> PSUM)
#           row-min straight off PSUM (DVE) || plain PSUM->bf16 drain (Scalar)
#           mx_i = ||t_i||^2 (STT); s_i = (mx-mn+eps)/8
#   simmn = sim - mn_i (bias-only Scalar passes)
#   sweep2: t1 = 8*ihat^T @ that - BIG*onehot   (8*cos + aligned mask, PSUM)
#           fused STT from PSUM: Mxe_i = sum_j max(s_i*t1, simmn)  (DVE)
#   identity: relu(s*t1 + mn - sim) = max(s*t1, sim-mn) - (sim-mn), so
#     sum_j 8*relu(cos-w)*s = Mxe_i - S_i + 4096*mn_i,  S_i = sum_j sim_ij
#     = t_i . T8 (T8 = sum_j f8(t_j), host). Aligned pairs: -BIG*s forces
#     max = simmn, cancelling exactly against -S+4096mn.
#   aligned part: G1 - sum_aligned cos; cos sum = Frobenius(U8, Vg)/8 with
#     U8 = oh_loc @ ihat8 (4 device matmuls), Vg = group sums of that (host).
#
# All inputs are host pre-blocked into their exact SBUF images so every DMA
# runs 2-4 KiB descriptors; the two big rhs tensors are split into 16 chunks
# across queues and issued from both SP and Act sequencers.
import numpy as np
import ml_dtypes

import concourse.mybir as mybir
import concourse.tile as tile
from concourse import bacc
from concourse.bass import ts

BF16 = mybir.dt.bfloat16
F32 = mybir.dt.float32
F8 = mybir.dt.float8e4
AF = mybir.ActivationFunctionType
OP = mybir.AluOpType
PM = mybir.MatmulPerfMode
nbf = ml_dtypes.bfloat16

N, D, G, NCORES = 4096, 512, 64, 8
L = N // NCORES            # 512 local rows per core
KT = D // 128              # 4 contraction chunks of 128
IT = L // 128              # 4 local i-tiles
JT = N // 512              # 8 j-tiles
BIG = 240.0                # exact in fp8-e4m3; dominates 8*cos
EPS_W = 1e-6

_CACHE = {}


def _build_program():
    nc = bacc.Bacc("TRN2", target_bir_lowering=False, debug=False,
                   enable_asserts=True, num_devices=NCORES)

    d_txt8 = nc.dram_tensor("txt8_blk", [128, JT * KT * 512], F8,
                            kind="ExternalInput").ap()
    d_that8 = nc.dram_tensor("that8_blk", [128, JT * KT * 512], F8,
                             kind="ExternalInput").ap()
    d_txt8_loc = nc.dram_tensor("txt8_loc_blk", [128, KT * L], F8,
                                kind="ExternalInput").ap()
    d_ihat8_T = nc.dram_tensor("ihat8_T_blk", [128, KT * L], F8,
                               kind="ExternalInput").ap()
    d_ihat8_rows = nc.dram_tensor("ihat8_rows_blk", [128, IT * D], F8,
                                  kind="ExternalInput").ap()
    d_txtloc = nc.dram_tensor("txtloc_blk", [128, IT * D], BF16,
                              kind="ExternalInput").ap()
    d_T8 = nc.dram_tensor("T8_bcast", [128, D], F32, kind="ExternalInput").ap()
    d_oh_iT = nc.dram_tensor("oh_iT_blk", [128, IT * G], F8,
                             kind="ExternalInput").ap()
    d_mask_lhsT = nc.dram_tensor("mask_lhsT_blk", [128, 2 * L], F8,
                                 kind="ExternalInput").ap()
    d_mask_rhs0 = nc.dram_tensor("mask_rhs0", [G, N], F8, kind="ExternalInput").ap()
    d_Vg = nc.dram_tensor("Vg", [G, D], BF16, kind="ExternalInput").ap()
    d_partials = nc.dram_tensor("partials", [128, 24], F32, kind="ExternalOutput").ap()

    with tile.TileContext(nc) as tc:
        with (
            tc.tile_pool(name="persist", bufs=1) as pp,
            tc.tile_pool(name="sims", bufs=IT) as psim,
            tc.tile_pool(name="simmns", bufs=2) as psmn,
            tc.tile_pool(name="small", bufs=1) as psm,
            tc.tile_pool(name="stats", bufs=2) as pst,
            tc.tile_pool(name="psum", bufs=2, space="PSUM") as pps,
        ):
            # -------- loads: critical tensors first, split SP/Act issue ----
            txt8_T_loc = pp.tile([128, KT * L], F8)
            nc.sync.dma_start(txt8_T_loc[:], d_txt8_loc)
            txt8_T_loc_v = txt8_T_loc[:].rearrange("p (c i) -> p c i", c=KT)

            # txt8 in (jt, c, j) blocked layout: 16 chunks, 2KB descriptors
            txt8 = pp.tile([128, JT * KT * 512], F8)
            txt8_v = txt8[:].rearrange("p (jt c j) -> p jt c j", jt=JT, c=KT)
            d_txt8_v = d_txt8.rearrange("p (jt c j) -> p jt c j", jt=JT, c=KT)
            for jt in range(JT):
                for ph in range(2):
                    sp = slice(64 * ph, 64 * ph + 64)
                    eng = nc.sync if ph == 0 else nc.scalar
                    eng.dma_start(txt8_v[sp, jt], d_txt8_v[sp, jt])

            txtloc_rows = pp.tile([128, IT * D], BF16)
            nc.sync.dma_start(txtloc_rows[:], d_txtloc)
            txtloc_v = txtloc_rows[:].rearrange("p (t d) -> p t d", t=IT)

            T8_sb = pp.tile([128, D], F32)
            nc.sync.dma_start(T8_sb[:], d_T8)

            ihat8_T = pp.tile([128, KT * L], F8)
            nc.scalar.dma_start(ihat8_T[:], d_ihat8_T)
            ihat8_T_v = ihat8_T[:].rearrange("p (c i) -> p c i", c=KT)

            ihat8_rows = pp.tile([128, IT * D], F8)
            nc.scalar.dma_start(ihat8_rows[:], d_ihat8_rows)
            ihat8_rows_v = ihat8_rows[:].rearrange("p (t d) -> p t d", t=IT)

            that8 = pp.tile([128, JT * KT * 512], F8)
            that8_v = that8[:].rearrange("p (jt c j) -> p jt c j", jt=JT, c=KT)
            d_that8_v = d_that8.rearrange("p (jt c j) -> p jt c j", jt=JT, c=KT)
            for jt in range(JT):
                for ph in range(2):
                    sp = slice(64 * ph, 64 * ph + 64)
                    nc.sync.dma_start(that8_v[sp, jt], d_that8_v[sp, jt])

            oh_iT = pp.tile([128, IT * G], F8)
            nc.sync.dma_start(oh_iT[:], d_oh_iT)
            oh_iT_v = oh_iT[:].rearrange("p (t g) -> p t g", t=IT)

            mask_lhsT = pp.tile([128, 2 * L], F8)
            nc.sync.dma_start(mask_lhsT[:], d_mask_lhsT)
            mask_lhsT_v = mask_lhsT[:].rearrange("p (b i) -> p b i", b=2)

            # mask rhs: zero the padded DR tile on Pool, DMA only the 64 real rows
            mask_rhsT = pp.tile([128, 2 * N], F8)
            nc.gpsimd.memset(mask_rhsT[:], 0.0)
            mask_rhsT_v = mask_rhsT[:].rearrange("p (b j) -> p b j", b=2)
            nc.sync.dma_start(mask_rhsT_v[0:G, 0], d_mask_rhs0)

            Vg_sb = pp.tile([G, D], BF16)
            nc.sync.dma_start(Vg_sb[:], d_Vg)

            partials = pp.tile([128, 24], F32)
            nc.gpsimd.memset(partials[:], 0.0)

            # ---------- early row stats (overlap with DMA) ----------
            mx = psm.tile([128, IT], F32)
            for t in range(IT):
                junk = pst.tile([128, D], BF16, tag="junk")
                nc.vector.scalar_tensor_tensor(
                    out=junk[:], in0=txtloc_v[:, t, :], scalar=1.0,
                    in1=txtloc_v[:, t, :], op0=OP.mult, op1=OP.mult,
                    accum_out=mx[:, t:t + 1])
            for t in range(IT):
                junk = pst.tile([128, D], BF16, tag="junk")
                nc.vector.scalar_tensor_tensor(
                    out=junk[:], in0=txtloc_v[:, t, :], scalar=1.0,
                    in1=T8_sb[:], op0=OP.mult, op1=OP.mult,
                    accum_out=partials[:, 8 + t:9 + t])

            # ---------------- sweep 1: sim, PSUM row-min || plain drain ----
            sim_panels = []
            mins32 = psm.tile([128, IT * 2], F32)
            mn32 = psm.tile([128, IT], F32)
            negmn = psm.tile([128, IT], F32)
            s32 = psm.tile([128, IT], F32)
            tmp32 = psm.tile([128, IT], F32)
            for it in range(IT):
                sim_sb = psim.tile([128, N], BF16, tag="sim")
                for h in range(2):
                    ps = pps.tile([128, 2048], F32, tag="mm")
                    for jq in range(4):
                        jt = 4 * h + jq
                        for cp in range(KT // 2):
                            nc.tensor.matmul(
                                ps[:, ts(jq, 512)],
                                txt8_T_loc_v[:, 2 * cp:2 * cp + 2, ts(it, 128)],
                                txt8_v[:, jt, 2 * cp:2 * cp + 2, :],
                                start=(cp == 0), stop=(cp == KT // 2 - 1),
                                perf_mode=PM.DoubleRow)
                    nc.vector.tensor_reduce(
                        out=mins32[:, it * 2 + h:it * 2 + h + 1], in_=ps[:],
                        axis=mybir.AxisListType.X, op=OP.min)
                    nc.scalar.activation(out=sim_sb[:, ts(h, 2048)], in_=ps[:],
                                         func=AF.Identity)
                nc.vector.tensor_reduce(
                    out=mn32[:, it:it + 1], in_=mins32[:, it * 2:it * 2 + 2],
                    axis=mybir.AxisListType.X, op=OP.min)
                nc.vector.tensor_scalar_mul(out=negmn[:, it:it + 1],
                                            in0=mn32[:, it:it + 1], scalar1=-1.0)
                nc.vector.tensor_tensor(out=tmp32[:, it:it + 1],
                                        in0=mx[:, it:it + 1],
                                        in1=mn32[:, it:it + 1], op=OP.subtract)
                nc.vector.tensor_scalar(out=s32[:, it:it + 1],
                                        in0=tmp32[:, it:it + 1],
                                        scalar1=EPS_W, scalar2=0.125,
                                        op0=OP.add, op1=OP.mult)
                sim_panels.append(sim_sb)

            # ---------------- U8 = oh_loc @ ihat8 ; Frobenius with Vg -------
            u8ps = pps.tile([128, 2048], F32, tag="mm")
            for t in range(IT):
                nc.tensor.matmul(u8ps[0:G, 0:D], oh_iT_v[:, t, :],
                                 ihat8_rows_v[:, t, :],
                                 start=(t == 0), stop=(t == IT - 1))
            junk64 = pst.tile([G, D], BF16, tag="junk64")
            nc.vector.scalar_tensor_tensor(
                out=junk64[:], in0=u8ps[0:G, 0:D], scalar=1.0,
                in1=Vg_sb[:], op0=OP.mult, op1=OP.mult,
                accum_out=partials[0:G, 20:21])

            # ---------------- sweep 2: masked 8cos, fused max-from-PSUM -----
            junkpan = pp.tile([128, N], BF16)
            for it in range(IT):
                simmn = psmn.tile([128, N], BF16, tag="simmn")
                for h in range(2):
                    nc.scalar.activation(out=simmn[:, ts(h, 2048)],
                                         in_=sim_panels[it][:, ts(h, 2048)],
                                         func=AF.Identity,
                                         bias=negmn[:, it:it + 1])
                for h in range(2):
                    ps = pps.tile([128, 2048], F32, tag="mm")
                    for jq in range(4):
                        jt = 4 * h + jq
                        nc.tensor.matmul(ps[:, ts(jq, 512)],
                                         mask_lhsT_v[:, :, ts(it, 128)],
                                         mask_rhsT_v[:, :, ts(jt, 512)],
                                         start=True, stop=False,
                                         perf_mode=PM.DoubleRow)
                    for jq in range(4):
                        jt = 4 * h + jq
                        for cp in range(KT // 2):
                            nc.tensor.matmul(
                                ps[:, ts(jq, 512)],
                                ihat8_T_v[:, 2 * cp:2 * cp + 2, ts(it, 128)],
                                that8_v[:, jt, 2 * cp:2 * cp + 2, :],
                                start=False, stop=(cp == KT // 2 - 1),
                                perf_mode=PM.DoubleRow)
                    # Mxe += sum_j max(s_i * t1, simmn), straight from PSUM
                    nc.vector.scalar_tensor_tensor(
                        out=junkpan[:, ts(h, 2048)], in0=ps[:],
                        scalar=s32[:, it:it + 1], in1=simmn[:, ts(h, 2048)],
                        op0=OP.mult, op1=OP.max,
                        accum_out=partials[:, 2 * it + h:2 * it + h + 1])

            # ship s and mn for the host-side reduction
            nc.vector.tensor_scalar_mul(out=partials[:, 12:16], in0=s32[:],
                                        scalar1=1.0)
            nc.vector.tensor_scalar_mul(out=partials[:, 16:20], in0=mn32[:],
                                        scalar1=1.0)

            nc.sync.dma_start(d_partials, partials[:])

    nc.compile()
    return nc


def _blk_T(a_T, inner):
    # [R*128, inner] -> [128, (r inner)]: partition-contiguous SBUF image
    r = a_T.shape[0] // 128
    return np.ascontiguousarray(
        a_T.reshape(r, 128, inner).transpose(1, 0, 2).reshape(128, -1))


def _blocked_big(a_T):
    # [D, N] -> [128, (jt c j)] where element (p, jt, c, j) = a_T[c*128+p, jt*512+j]
    return np.ascontiguousarray(
        a_T.reshape(KT, 128, JT, 512).transpose(1, 2, 0, 3).reshape(128, -1))


def _host_in_maps(image_features, text_features, instr_d):
    nf8 = mybir.dt.np(F8)
    img = np.asarray(image_features, np.float32)
    txt = np.asarray(text_features, np.float32)
    ins = np.asarray(instr_d)
    oh = (ins[None, :] == np.arange(G, dtype=ins.dtype)[:, None]).astype(np.float32)

    tn = np.sqrt((txt * txt).sum(1))
    inorm = np.sqrt((img * img).sum(1))
    that = txt / tn[:, None]
    ihat8 = (8.0 / inorm[:, None]) * img

    txt8 = txt.astype(nf8)
    txt8_T = np.ascontiguousarray(txt8.T)
    txt8_blk = _blocked_big(txt8_T)
    that8_blk = _blocked_big(that.astype(nf8).T)
    ihat8_f8 = ihat8.astype(nf8)
    ihat8_T = np.ascontiguousarray(ihat8_f8.T)
    T8 = txt8.astype(np.float32).sum(0)                      # [D]
    T8_bcast = np.ascontiguousarray(
        np.broadcast_to(T8[None, :], (128, D)), dtype=np.float32)
    Vg = (oh @ that).astype(nbf)                             # [G, D]
    oh_iT_full = np.ascontiguousarray(oh.T).astype(nf8)
    mask_rhs0 = oh.astype(nf8)

    in_maps = []
    for c in range(NCORES):
        sl = slice(c * L, (c + 1) * L)
        mask_lhsT = np.zeros((256, L), nf8)
        mask_lhsT[0:G] = (-BIG * oh[:, sl]).astype(nf8)
        in_maps.append({
            "txt8_blk": txt8_blk,
            "txt8_loc_blk": _blk_T(np.ascontiguousarray(txt8_T[:, sl]), L),
            "that8_blk": that8_blk,
            "ihat8_T_blk": _blk_T(np.ascontiguousarray(ihat8_T[:, sl]), L),
            "ihat8_rows_blk": _blk_T(np.ascontiguousarray(ihat8_f8[sl]), D),
            "txtloc_blk": _blk_T(np.ascontiguousarray(txt[sl].astype(nbf)), D),
            "T8_bcast": T8_bcast,
            "oh_iT_blk": _blk_T(np.ascontiguousarray(oh_iT_full[sl]), G),
            "mask_lhsT_blk": _blk_T(mask_lhsT, L),
            "mask_rhs0": mask_rhs0,
            "Vg": Vg,
        })
    return in_maps


def _reduce(partials_per_core, ins):
    ngg = (ins[None, :] == np.arange(G, dtype=ins.dtype)[:, None]).sum(1)  # [G]
    total = np.float64(0.0)
    for c, p in enumerate(partials_per_core):
        p = np.asarray(p, np.float64)
        ngl = (ins[c * L:(c + 1) * L][None, :]
               == np.arange(G, dtype=ins.dtype)[:, None]).sum(1)  # [G]
        Mxe = p[:, 0:8].reshape(128, 4, 2).sum(2)                # [128, IT]
        S = p[:, 8:12]
        s = p[:, 12:16]
        mn = p[:, 16:20]
        F8sum = p[0:G, 20].sum()
        relu8 = ((Mxe - S + N * mn) / s).sum()
        g1 = np.float64((ngl * ngg).sum())
        total += relu8 / 8.0 + g1 - F8sum / 8.0
    return np.float32(total / (N * N))


def kernel(**inputs) -> np.ndarray:
    from concourse.bass_utils import run_bass_kernel_spmd

    if "nc" not in _CACHE:
        _CACHE["nc"] = _build_program()
    nc = _CACHE["nc"]
    in_maps = _host_in_maps(**inputs)
    res = run_bass_kernel_spmd(nc, in_maps, core_ids=list(range(NCORES)),
                               trace=False)
    _CACHE["last_res"] = res

    ins = np.asarray(inputs["instr_d"])
    return _reduce([r["partials"] for r in res.results], ins)
